# revision 9
# baseline (speedup 1.0000x reference)
"""Trainium2 Bass kernel for additive-attention (nn_Attention_77403900609148).

Computation (per batch row b):
    inp  = input @ W_in.T + b_in                      # [H]
    ctx  = W_ctx @ context[b].T + b_ctx               # [H, S]
    att  = V . tanh(inp[:,None] + ctx)                # [S]
    att  = att * alpha_mask[b]   (or -inf mask branch, resolved host-side)
    alpha = softmax(att)                              # [S]
    hidden = ctx @ alpha = W_ctx @ (context[b].T @ alpha) + b_ctx

Device computes exp(att) (softmax without max-subtraction: energies are
bounded ~|att| < 40, exp is safe in fp32) and the unnormalized
y = context.T @ exp(att); the 1/sum(exp) normalization and b_ctx bias are
applied host-side.

Sharding: data-parallel over batch B=64 across 8 NeuronCores (8 rows each);
the small weights are replicated.
"""

import sys

if '/opt/trn_rl_repo' not in sys.path:
    sys.path.insert(0, '/opt/trn_rl_repo')

import numpy as np

import concourse.bass as bass
import concourse.tile as tile
from concourse import mybir, bacc, masks
from concourse.bass_utils import run_bass_kernel_spmd

F32 = mybir.dt.float32
F32R = mybir.dt.float32r
AFT = mybir.ActivationFunctionType

B, S, D, H = 64, 2048, 512, 512
NCORES = 8
BLOC = B // NCORES          # 8 batch rows per core
SC = S // 128               # 16 s-chunks of 128
ST = S // 512               # 4 s-tiles of 512
DC = D // 128               # 4 d-chunks
HC = H // 128               # 4 h-chunks


def build_program(inf_branch: bool):
    nc = bacc.Bacc("TRN2", num_devices=1, debug=False, target_bir_lowering=False)

    # ---- per-core DRAM I/O ----
    d_input = nc.dram_tensor("input_l", [BLOC, D], F32, kind="ExternalInput")
    d_ctx = nc.dram_tensor("context_l", [BLOC, S, D], F32R, kind="ExternalInput")
    d_m1 = nc.dram_tensor("m1", [BLOC, S], F32, kind="ExternalInput")
    if inf_branch:
        d_m2 = nc.dram_tensor("m2", [BLOC, S], F32, kind="ExternalInput")
    d_wctx = nc.dram_tensor("w_ctx", [H, D], F32R, kind="ExternalInput")
    d_win = nc.dram_tensor("w_in", [H, D], F32R, kind="ExternalInput")
    d_v = nc.dram_tensor("vvec", [H], F32R, kind="ExternalInput")
    d_bsum = nc.dram_tensor("bsum", [H], F32, kind="ExternalInput")   # b_in + b_ctx
    d_ident = nc.dram_tensor("ident_i", [128, 128], F32, kind="ExternalInput")

    d_hid = nc.dram_tensor("hidden_o", [BLOC, H], F32, kind="ExternalOutput")
    d_alpha = nc.dram_tensor("alpha_o", [BLOC, S], F32, kind="ExternalOutput")
    d_lsum = nc.dram_tensor("lsum_o", [BLOC, ST], F32, kind="ExternalOutput")

    with tile.TileContext(nc) as tc:
        with (
            tc.tile_pool(name="persist", bufs=1) as pp,
            tc.tile_pool(name="cnat", bufs=2) as cnat_pool,
            tc.tile_pool(name="ctp", bufs=2) as ct_pool,
            tc.tile_pool(name="tpool", bufs=3) as t_pool,
            tc.tile_pool(name="rowp", bufs=2) as row_pool,
            tc.tile_pool(name="ps_main", bufs=2, space="PSUM") as ps_main,
            tc.tile_pool(name="ps_tr", bufs=2, space="PSUM") as ps_tr,
            tc.tile_pool(name="ps_small", bufs=1, space="PSUM") as ps_small,
            tc.tile_pool(name="ps_att", bufs=2, space="PSUM") as ps_att,
            tc.tile_pool(name="ps_y", bufs=1, space="PSUM") as ps_y,
        ):
            # ---------------- setup ----------------
            ident_f = pp.tile([128, 128], F32, tag="ident_f")
            nc.sync.dma_start(ident_f[:], d_ident.ap())
            ident = pp.tile([128, 128], F32R, tag="ident")
            nc.sync.dma_start(ident[:], d_ident.ap().bitcast(F32R))

            v_sb = pp.tile([128, HC], F32R, tag="v_sb")
            nc.sync.dma_start(v_sb[:], d_v.ap().rearrange("(c p) -> p c", p=128))
            bsum_sb = pp.tile([128, HC], F32, tag="bsum_sb")
            nc.sync.dma_start(bsum_sb[:], d_bsum.ap().rearrange("(c p) -> p c", p=128))

            # W_ctx natural + transpose -> W_ctxT[j] = [d-chunk j (128), h (512)]
            wctxT = [pp.tile([128, H], F32R, tag=f"wctxT{j}", name=f"wctxT{j}")
                     for j in range(DC)]
            with tc.tile_pool(name="setup", bufs=1) as sp:
                winT = [sp.tile([128, H], F32R, tag=f"winT{j}", name=f"winT{j}")
                        for j in range(DC)]
                for (dram, dstT, nm) in ((d_wctx, wctxT, "wc"), (d_win, winT, "wi")):
                    wnat = sp.tile([128, HC, D], F32R, tag="wnat", name=f"wnat{nm}")
                    nc.sync.dma_start(
                        wnat[:], dram.ap().rearrange("(c p) d -> p c d", p=128))
                    for j in range(DC):
                        pw = ps_tr.tile([128, 512], F32R, tag="ps_tr")
                        for c in range(HC):
                            nc.tensor.transpose(
                                pw[:, c * 128:(c + 1) * 128],
                                wnat[:, c, j * 128:(j + 1) * 128], ident[:])
                        nc.vector.tensor_copy(dstT[j][:], pw[:])

                # input_linear: inp_T[h, b] = W_in @ input.T  (+ bsum bias)
                in_sb = sp.tile([BLOC, D], F32, tag="in_sb")
                nc.sync.dma_start(in_sb[:], d_input.ap())
                p_it = ps_small.tile([128, DC * BLOC], F32, tag="ps_small")
                for j in range(DC):
                    nc.tensor.transpose(
                        p_it[:, j * BLOC:(j + 1) * BLOC],
                        in_sb[:, j * 128:(j + 1) * 128], ident_f[:BLOC, :BLOC])
                inputT = sp.tile([128, DC, BLOC], F32R, tag="inputT")
                nc.vector.tensor_copy(
                    inputT[:], p_it[:].rearrange("p (j b) -> p j b", j=DC))

                inpb = [pp.tile([128, BLOC], F32, tag=f"inpb{c}", name=f"inpb{c}")
                        for c in range(HC)]
                for c in range(HC):
                    p_inp = ps_small.tile([128, BLOC], F32, tag="ps_small")
                    for j in range(DC):
                        nc.tensor.matmul(
                            p_inp[:], winT[j][:, c * 128:(c + 1) * 128],
                            inputT[:, j, :], start=(j == 0), stop=(j == DC - 1))
                    nc.vector.tensor_scalar_add(
                        inpb[c][:], p_inp[:], bsum_sb[:, c:c + 1])

            yT_all = pp.tile([128, DC, BLOC], F32R, tag="yT_all")

            # ---------------- per batch ----------------
            for b in range(BLOC):
                c_nat = cnat_pool.tile([128, SC, D], F32R, tag="c_nat")
                ctx_b = d_ctx.ap()[b].rearrange("(n p) d -> p n d", p=128)
                for q in range(4):
                    nc.sync.dma_start(c_nat[:, q * 4:(q + 1) * 4, :],
                                      ctx_b[:, q * 4:(q + 1) * 4, :])
                m1_sb = row_pool.tile([1, S], F32, tag="m1_sb")
                nc.sync.dma_start(m1_sb[:], d_m1.ap()[b:b + 1, :])
                if inf_branch:
                    m2_sb = row_pool.tile([1, S], F32, tag="m2_sb")
                    nc.sync.dma_start(m2_sb[:], d_m2.ap()[b:b + 1, :])

                # transpose context: CT[p=d, (j, s)] ; j = d-chunk
                ct = ct_pool.tile([128, DC, S], F32R, tag="ct")
                for sc in range(SC):
                    ptr = ps_tr.tile([128, 512], F32R, tag="ps_tr")
                    for j in range(DC):
                        nc.tensor.transpose(
                            ptr[:, j * 128:(j + 1) * 128],
                            c_nat[:, sc, j * 128:(j + 1) * 128], ident[:])
                    nc.vector.tensor_copy(
                        ct[:, :, sc * 128:(sc + 1) * 128],
                        ptr[:].rearrange("p (j q) -> p j q", j=DC))

                alpha_row = row_pool.tile([1, S], F32, tag="alpha_row")
                acc4 = row_pool.tile([1, ST], F32, tag="acc4")
                p_y = ps_y.tile([1, D], F32, tag="ps_y")

                p_atts = {}

                def st_head(st):
                    p_att = ps_att.tile([1, 512], F32, tag="ps_att",
                                        name=f"p_att_{b}_{st}")
                    p_atts[st] = p_att
                    for h in range(HC):
                        p_main = ps_main.tile([128, 512], F32, tag="ps_main",
                                              name=f"p_main_{b}_{st}_{h}")
                        for j in range(DC):
                            nc.tensor.matmul(
                                p_main[:], wctxT[j][:, h * 128:(h + 1) * 128],
                                ct[:, j, st * 512:(st + 1) * 512],
                                start=(j == 0), stop=(j == DC - 1))
                        t_sb = t_pool.tile([128, 512], F32R, tag="t_sb",
                                           name=f"t_sb_{b}_{st}_{h}")
                        nc.scalar.activation(
                            t_sb[:], p_main[:], AFT.Tanh, bias=inpb[h][:, b:b + 1])
                        nc.tensor.matmul(
                            p_att[:], v_sb[:, h:h + 1], t_sb[:],
                            start=(h == 0), stop=(h == HC - 1))

                def st_tail(st):
                    p_att = p_atts.pop(st)
                    att2 = row_pool.tile([1, 512], F32, tag="att2",
                                         name=f"att2_{b}_{st}")
                    nc.vector.tensor_mul(
                        att2[:], p_att[:], m1_sb[:, st * 512:(st + 1) * 512])
                    if inf_branch:
                        nc.vector.tensor_add(
                            att2[:], att2[:], m2_sb[:, st * 512:(st + 1) * 512])
                    nc.scalar.activation(
                        alpha_row[:, st * 512:(st + 1) * 512], att2[:], AFT.Exp,
                        accum_out=acc4[:, st:st + 1])
                    p_at4 = ps_small.tile([128, 4], F32, tag="ps_small",
                                          name=f"p_at4_{b}_{st}")
                    for q in range(4):
                        sc = st * 4 + q
                        nc.tensor.transpose(
                            p_at4[:, q:q + 1],
                            alpha_row[:, sc * 128:(sc + 1) * 128],
                            ident_f[:1, :1])
                    alphaT = row_pool.tile([128, 4], F32R, tag="alphaT",
                                           name=f"alphaT_{b}_{st}")
                    nc.vector.tensor_copy(alphaT[:], p_at4[:])
                    for q in range(4):
                        sc = st * 4 + q
                        nc.tensor.matmul(
                            p_y[:], alphaT[:, q:q + 1], c_nat[:, sc, :],
                            start=(sc == 0), stop=(sc == SC - 1))

                for st in range(ST):
                    st_head(st)
                    if st > 0:
                        st_tail(st - 1)
                st_tail(ST - 1)

                # outputs: unnormalized exp(att) row, its partial sums, and y
                nc.sync.dma_start(d_alpha.ap()[b:b + 1, :], alpha_row[:])
                nc.sync.dma_start(d_lsum.ap()[b:b + 1, :], acc4[:])

                y_sb = row_pool.tile([1, D], F32, tag="y_sb")
                nc.vector.tensor_copy(y_sb[:], p_y[:])
                p_yt = ps_small.tile([128, DC], F32, tag="ps_small")
                for j in range(DC):
                    nc.tensor.transpose(
                        p_yt[:, j:j + 1],
                        y_sb[:, j * 128:(j + 1) * 128], ident_f[:1, :1])
                nc.vector.tensor_copy(yT_all[:, :, b], p_yt[:])

            # ---------------- hidden_unnorm = W_ctx @ y ----------------
            hid_ps_out = ps_tr.tile([BLOC, H], F32, tag="ps_tr")
            for c in range(HC):
                p_hid = ps_small.tile([128, BLOC], F32, tag="ps_small")
                for j in range(DC):
                    nc.tensor.matmul(
                        p_hid[:], wctxT[j][:, c * 128:(c + 1) * 128],
                        yT_all[:, j, :], start=(j == 0), stop=(j == DC - 1))
                hidT = row_pool.tile([128, BLOC], F32, tag="hidT")
                nc.vector.tensor_copy(hidT[:], p_hid[:])
                nc.tensor.transpose(
                    hid_ps_out[:, c * 128:(c + 1) * 128], hidT[:], ident_f[:])
            hid_out = row_pool.tile([BLOC, H], F32, tag="hid_out")
            nc.vector.tensor_copy(hid_out[:], hid_ps_out[:])
            nc.sync.dma_start(d_hid.ap(), hid_out[:])

    nc.compile()
    return nc


_PROG_CACHE = {}
_IDENT = np.eye(128, dtype=np.float32)


def _get_prog(inf_branch: bool):
    if inf_branch not in _PROG_CACHE:
        _PROG_CACHE[inf_branch] = build_program(inf_branch)
    return _PROG_CACHE[inf_branch]


def make_in_maps(input, context, alpha_mask, W_in, b_in, W_ctx, b_ctx, V, mask):
    input = np.ascontiguousarray(input, dtype=np.float32)
    context = np.ascontiguousarray(context, dtype=np.float32)
    alpha_mask = np.ascontiguousarray(alpha_mask, dtype=np.float32)
    W_in = np.ascontiguousarray(W_in, dtype=np.float32)
    W_ctx = np.ascontiguousarray(W_ctx, dtype=np.float32)
    V = np.ascontiguousarray(V, dtype=np.float32)
    b_in = np.ascontiguousarray(b_in, dtype=np.float32)
    b_ctx = np.ascontiguousarray(b_ctx, dtype=np.float32)

    mb = np.asarray(mask) != 0
    n_true = int(mb.sum())
    inf_branch = (n_true > 0) and (n_true == S)
    if inf_branch:
        m1 = np.ones_like(alpha_mask)
        m2 = np.where(mb, np.float32(-1e30), np.float32(0.0)).astype(np.float32)
    else:
        m1 = alpha_mask
        m2 = None

    bsum = (b_in + b_ctx).astype(np.float32)
    global _IDENT
    in_maps = []
    for g in range(NCORES):
        sl = slice(g * BLOC, (g + 1) * BLOC)
        m = {
            "input_l": input[sl],
            "context_l": context[sl],
            "m1": m1[sl],
            "w_ctx": W_ctx,
            "w_in": W_in,
            "vvec": V,
            "bsum": bsum,
            "ident_i": _IDENT,
        }
        if inf_branch:
            m["m2"] = m2[sl]
        in_maps.append(m)
    return in_maps, inf_branch


def assemble_outputs(res, b_ctx):
    hid, alp = [], []
    for g in range(NCORES):
        r = res.results[g]
        L = r["lsum_o"].sum(axis=1, keepdims=True)          # [BLOC, 1]
        alp.append(r["alpha_o"] / L)
        hid.append(r["hidden_o"] / L + b_ctx[None, :])
    return (np.concatenate(hid, axis=0).astype(np.float32),
            np.concatenate(alp, axis=0).astype(np.float32))


def kernel(**inputs):
    in_maps, inf_branch = make_in_maps(**inputs)
    nc = _get_prog(inf_branch)
    res = run_bass_kernel_spmd(nc, in_maps, core_ids=list(range(NCORES)))
    b_ctx = np.ascontiguousarray(inputs["b_ctx"], dtype=np.float32)
    return assemble_outputs(res, b_ctx)


# revision 11
# speedup vs baseline: 1.0087x; 1.0087x over previous
"""Trainium2 Bass kernel for additive-attention (nn_Attention_77403900609148).

Computation (per batch row b):
    inp  = input @ W_in.T + b_in                      # [H]
    ctx  = W_ctx @ context[b].T + b_ctx               # [H, S]
    att  = V . tanh(inp[:,None] + ctx)                # [S]
    att  = att * alpha_mask[b]   (or -inf mask branch, resolved host-side)
    alpha = softmax(att)                              # [S]
    hidden = ctx @ alpha = W_ctx @ (context[b].T @ alpha) + b_ctx

Device computes exp(att) (softmax without max-subtraction: energies are
bounded ~|att| < 40, exp is safe in fp32) and the unnormalized
y = context.T @ exp(att); the 1/sum(exp) normalization and b_ctx bias are
applied host-side.

Sharding: data-parallel over batch B=64 across 8 NeuronCores (8 rows each);
the small weights are replicated.
"""

import sys

if '/opt/trn_rl_repo' not in sys.path:
    sys.path.insert(0, '/opt/trn_rl_repo')

import numpy as np

import concourse.bass as bass
import concourse.tile as tile
from concourse import mybir, bacc, masks
from concourse.bass_utils import run_bass_kernel_spmd

F32 = mybir.dt.float32
F32R = mybir.dt.float32r
AFT = mybir.ActivationFunctionType

B, S, D, H = 64, 2048, 512, 512
NCORES = 8
BLOC = B // NCORES          # 8 batch rows per core
SC = S // 128               # 16 s-chunks of 128
ST = S // 512               # 4 s-tiles of 512
DC = D // 128               # 4 d-chunks
HC = H // 128               # 4 h-chunks


def build_program(inf_branch: bool):
    nc = bacc.Bacc("TRN2", num_devices=1, debug=False, target_bir_lowering=False)

    # ---- per-core DRAM I/O ----
    d_input = nc.dram_tensor("input_l", [BLOC, D], F32, kind="ExternalInput")
    d_ctx = nc.dram_tensor("context_l", [BLOC, S, D], F32R, kind="ExternalInput")
    d_m1 = nc.dram_tensor("m1", [BLOC, S], F32, kind="ExternalInput")
    if inf_branch:
        d_m2 = nc.dram_tensor("m2", [BLOC, S], F32, kind="ExternalInput")
    d_wctx = nc.dram_tensor("w_ctx", [H, D], F32R, kind="ExternalInput")
    d_win = nc.dram_tensor("w_in", [H, D], F32R, kind="ExternalInput")
    d_v = nc.dram_tensor("vvec", [H], F32R, kind="ExternalInput")
    d_bsum = nc.dram_tensor("bsum", [H], F32, kind="ExternalInput")   # b_in + b_ctx
    d_ident = nc.dram_tensor("ident_i", [128, 128], F32, kind="ExternalInput")

    d_hid = nc.dram_tensor("hidden_o", [BLOC, H], F32, kind="ExternalOutput")
    d_alpha = nc.dram_tensor("alpha_o", [BLOC, S], F32, kind="ExternalOutput")
    d_lsum = nc.dram_tensor("lsum_o", [BLOC, ST], F32, kind="ExternalOutput")

    with tile.TileContext(nc) as tc:
        with (
            tc.tile_pool(name="persist", bufs=1) as pp,
            tc.tile_pool(name="cnat", bufs=2) as cnat_pool,
            tc.tile_pool(name="ctp", bufs=1) as ct_pool,
            tc.tile_pool(name="tpool", bufs=3) as t_pool,
            tc.tile_pool(name="rowp", bufs=2) as row_pool,
            tc.tile_pool(name="ps_main", bufs=2, space="PSUM") as ps_main,
            tc.tile_pool(name="ps_tr", bufs=2, space="PSUM") as ps_tr,
            tc.tile_pool(name="ps_small", bufs=1, space="PSUM") as ps_small,
            tc.tile_pool(name="ps_att", bufs=2, space="PSUM") as ps_att,
            tc.tile_pool(name="ps_y", bufs=1, space="PSUM") as ps_y,
        ):
            # ---------------- setup ----------------
            # DMA order matters: everything the first tanh depends on
            # (ident, W_in, input, biases, W_ctx, context[0]) goes first.
            ident_f = pp.tile([128, 128], F32, tag="ident_f")
            nc.sync.dma_start(ident_f[:], d_ident.ap())
            ident = pp.tile([128, 128], F32R, tag="ident")
            nc.sync.dma_start(ident[:], d_ident.ap().bitcast(F32R))

            sp_ctx = tc.tile_pool(name="setup", bufs=1)
            sp = sp_ctx.__enter__()
            winT = [sp.tile([128, H], F32R, tag=f"winT{j}", name=f"winT{j}")
                    for j in range(DC)]
            wnat_wi = sp.tile([128, HC, D], F32R, tag="wnat", name="wnat_wi")
            nc.sync.dma_start(
                wnat_wi[:], d_win.ap().rearrange("(c p) d -> p c d", p=128))
            in_sb = sp.tile([BLOC, D], F32, tag="in_sb")
            nc.sync.dma_start(in_sb[:], d_input.ap())
            v_sb = pp.tile([128, HC], F32R, tag="v_sb")
            nc.sync.dma_start(v_sb[:], d_v.ap().rearrange("(c p) -> p c", p=128))
            bsum_sb = pp.tile([128, HC], F32, tag="bsum_sb")
            nc.sync.dma_start(bsum_sb[:], d_bsum.ap().rearrange("(c p) -> p c", p=128))

            wctxT = [pp.tile([128, H], F32R, tag=f"wctxT{j}", name=f"wctxT{j}")
                     for j in range(DC)]
            wnat_wc = pp.tile([128, HC, D], F32R, tag="wnat_wc")
            nc.sync.dma_start(
                wnat_wc[:], d_wctx.ap().rearrange("(c p) d -> p c d", p=128))

            # batch 0 context + m1 loads, queued right behind the weights
            c_nat0 = cnat_pool.tile([128, SC, D], F32R, tag="c_nat", name="c_nat0")
            ctx_b0 = d_ctx.ap()[0].rearrange("(n p) d -> p n d", p=128)
            for q in range(4):
                nc.sync.dma_start(c_nat0[:, q * 4:(q + 1) * 4, :],
                                  ctx_b0[:, q * 4:(q + 1) * 4, :])
            m1_sb0 = row_pool.tile([1, S], F32, tag="m1_sb", name="m1_sb0")
            nc.sync.dma_start(m1_sb0[:], d_m1.ap()[0:1, :])

            # W_in transposes + input_linear (gates the first tanh)
            for j in range(DC):
                pw = ps_tr.tile([128, 512], F32R, tag="ps_tr", name=f"pw_wi{j}")
                for c in range(HC):
                    nc.tensor.transpose(
                        pw[:, c * 128:(c + 1) * 128],
                        wnat_wi[:, c, j * 128:(j + 1) * 128], ident[:])
                nc.vector.tensor_copy(winT[j][:], pw[:])

            p_it = ps_small.tile([128, DC * BLOC], F32, tag="ps_small")
            for j in range(DC):
                nc.tensor.transpose(
                    p_it[:, j * BLOC:(j + 1) * BLOC],
                    in_sb[:, j * 128:(j + 1) * 128], ident_f[:BLOC, :BLOC])
            inputT = sp.tile([128, DC, BLOC], F32R, tag="inputT")
            nc.vector.tensor_copy(
                inputT[:], p_it[:].rearrange("p (j b) -> p j b", j=DC))

            inpb = [pp.tile([128, BLOC], F32, tag=f"inpb{c}", name=f"inpb{c}")
                    for c in range(HC)]
            for c in range(HC):
                p_inp = ps_small.tile([128, BLOC], F32, tag="ps_small")
                for j in range(DC):
                    nc.tensor.matmul(
                        p_inp[:], winT[j][:, c * 128:(c + 1) * 128],
                        inputT[:, j, :], start=(j == 0), stop=(j == DC - 1))
                nc.vector.tensor_scalar_add(
                    inpb[c][:], p_inp[:], bsum_sb[:, c:c + 1])
            sp_ctx.__exit__(None, None, None)

            # W_ctx transposes
            for j in range(DC):
                pw = ps_tr.tile([128, 512], F32R, tag="ps_tr", name=f"pw_wc{j}")
                for c in range(HC):
                    nc.tensor.transpose(
                        pw[:, c * 128:(c + 1) * 128],
                        wnat_wc[:, c, j * 128:(j + 1) * 128], ident[:])
                nc.vector.tensor_copy(wctxT[j][:], pw[:])

            yT_all = pp.tile([128, DC, BLOC], F32R, tag="yT_all")

            # ---------------- per batch ----------------
            for b in range(BLOC):
                if b == 0:
                    c_nat = c_nat0
                    m1_sb = m1_sb0
                else:
                    c_nat = cnat_pool.tile([128, SC, D], F32R, tag="c_nat",
                                           name=f"c_nat{b}")
                    ctx_b = d_ctx.ap()[b].rearrange("(n p) d -> p n d", p=128)
                    for q in range(4):
                        nc.sync.dma_start(c_nat[:, q * 4:(q + 1) * 4, :],
                                          ctx_b[:, q * 4:(q + 1) * 4, :])
                    m1_sb = row_pool.tile([1, S], F32, tag="m1_sb",
                                          name=f"m1_sb{b}")
                    nc.sync.dma_start(m1_sb[:], d_m1.ap()[b:b + 1, :])
                if inf_branch:
                    m2_sb = row_pool.tile([1, S], F32, tag="m2_sb")
                    nc.sync.dma_start(m2_sb[:], d_m2.ap()[b:b + 1, :])

                # transpose context: CT[p=d, (j, s)] ; j = d-chunk
                ct = ct_pool.tile([128, DC, S], F32R, tag="ct")
                for sc in range(SC):
                    ptr = ps_tr.tile([128, 512], F32R, tag="ps_tr")
                    for j in range(DC):
                        nc.tensor.transpose(
                            ptr[:, j * 128:(j + 1) * 128],
                            c_nat[:, sc, j * 128:(j + 1) * 128], ident[:])
                    nc.vector.tensor_copy(
                        ct[:, :, sc * 128:(sc + 1) * 128],
                        ptr[:].rearrange("p (j q) -> p j q", j=DC))

                alpha_row = row_pool.tile([1, S], F32, tag="alpha_row")
                acc4 = row_pool.tile([1, ST], F32, tag="acc4")
                p_y = ps_y.tile([1, D], F32, tag="ps_y")

                p_atts = {}

                def st_head(st):
                    p_att = ps_att.tile([1, 512], F32, tag="ps_att",
                                        name=f"p_att_{b}_{st}")
                    p_atts[st] = p_att
                    for h in range(HC):
                        p_main = ps_main.tile([128, 512], F32, tag="ps_main",
                                              name=f"p_main_{b}_{st}_{h}")
                        for j in range(DC):
                            nc.tensor.matmul(
                                p_main[:], wctxT[j][:, h * 128:(h + 1) * 128],
                                ct[:, j, st * 512:(st + 1) * 512],
                                start=(j == 0), stop=(j == DC - 1))
                        t_sb = t_pool.tile([128, 512], F32R, tag="t_sb",
                                           name=f"t_sb_{b}_{st}_{h}")
                        nc.scalar.activation(
                            t_sb[:], p_main[:], AFT.Tanh, bias=inpb[h][:, b:b + 1])
                        nc.tensor.matmul(
                            p_att[:], v_sb[:, h:h + 1], t_sb[:],
                            start=(h == 0), stop=(h == HC - 1))

                def st_tail(st):
                    p_att = p_atts.pop(st)
                    att2 = row_pool.tile([1, 512], F32, tag="att2",
                                         name=f"att2_{b}_{st}")
                    nc.vector.tensor_mul(
                        att2[:], p_att[:], m1_sb[:, st * 512:(st + 1) * 512])
                    if inf_branch:
                        nc.vector.tensor_add(
                            att2[:], att2[:], m2_sb[:, st * 512:(st + 1) * 512])
                    nc.scalar.activation(
                        alpha_row[:, st * 512:(st + 1) * 512], att2[:], AFT.Exp,
                        accum_out=acc4[:, st:st + 1])
                    p_at4 = ps_small.tile([128, 4], F32, tag="ps_small",
                                          name=f"p_at4_{b}_{st}")
                    for q in range(4):
                        sc = st * 4 + q
                        nc.tensor.transpose(
                            p_at4[:, q:q + 1],
                            alpha_row[:, sc * 128:(sc + 1) * 128],
                            ident_f[:1, :1])
                    alphaT = row_pool.tile([128, 4], F32R, tag="alphaT",
                                           name=f"alphaT_{b}_{st}")
                    nc.vector.tensor_copy(alphaT[:], p_at4[:])
                    for q in range(4):
                        sc = st * 4 + q
                        nc.tensor.matmul(
                            p_y[:], alphaT[:, q:q + 1], c_nat[:, sc, :],
                            start=(sc == 0), stop=(sc == SC - 1))

                for st in range(ST):
                    st_head(st)
                    if st > 0:
                        st_tail(st - 1)
                st_tail(ST - 1)

                # outputs: unnormalized exp(att) row, its partial sums, and y
                nc.sync.dma_start(d_alpha.ap()[b:b + 1, :], alpha_row[:])
                nc.sync.dma_start(d_lsum.ap()[b:b + 1, :], acc4[:])

                y_sb = row_pool.tile([1, D], F32, tag="y_sb")
                nc.vector.tensor_copy(y_sb[:], p_y[:])
                p_yt = ps_small.tile([128, DC], F32, tag="ps_small")
                for j in range(DC):
                    nc.tensor.transpose(
                        p_yt[:, j:j + 1],
                        y_sb[:, j * 128:(j + 1) * 128], ident_f[:1, :1])
                nc.vector.tensor_copy(yT_all[:, :, b], p_yt[:])

            # ---------------- hidden_unnorm = W_ctx @ y ----------------
            hid_ps_out = ps_tr.tile([BLOC, H], F32, tag="ps_tr")
            for c in range(HC):
                p_hid = ps_small.tile([128, BLOC], F32, tag="ps_small")
                for j in range(DC):
                    nc.tensor.matmul(
                        p_hid[:], wctxT[j][:, c * 128:(c + 1) * 128],
                        yT_all[:, j, :], start=(j == 0), stop=(j == DC - 1))
                hidT = row_pool.tile([128, BLOC], F32, tag="hidT")
                nc.vector.tensor_copy(hidT[:], p_hid[:])
                nc.tensor.transpose(
                    hid_ps_out[:, c * 128:(c + 1) * 128], hidT[:], ident_f[:])
            hid_out = row_pool.tile([BLOC, H], F32, tag="hid_out")
            nc.vector.tensor_copy(hid_out[:], hid_ps_out[:])
            nc.sync.dma_start(d_hid.ap(), hid_out[:])

    nc.compile()
    return nc


_PROG_CACHE = {}
_IDENT = np.eye(128, dtype=np.float32)


def _get_prog(inf_branch: bool):
    if inf_branch not in _PROG_CACHE:
        _PROG_CACHE[inf_branch] = build_program(inf_branch)
    return _PROG_CACHE[inf_branch]


def make_in_maps(input, context, alpha_mask, W_in, b_in, W_ctx, b_ctx, V, mask):
    input = np.ascontiguousarray(input, dtype=np.float32)
    context = np.ascontiguousarray(context, dtype=np.float32)
    alpha_mask = np.ascontiguousarray(alpha_mask, dtype=np.float32)
    W_in = np.ascontiguousarray(W_in, dtype=np.float32)
    W_ctx = np.ascontiguousarray(W_ctx, dtype=np.float32)
    V = np.ascontiguousarray(V, dtype=np.float32)
    b_in = np.ascontiguousarray(b_in, dtype=np.float32)
    b_ctx = np.ascontiguousarray(b_ctx, dtype=np.float32)

    mb = np.asarray(mask) != 0
    n_true = int(mb.sum())
    inf_branch = (n_true > 0) and (n_true == S)
    if inf_branch:
        m1 = np.ones_like(alpha_mask)
        m2 = np.where(mb, np.float32(-1e30), np.float32(0.0)).astype(np.float32)
    else:
        m1 = alpha_mask
        m2 = None

    bsum = (b_in + b_ctx).astype(np.float32)
    global _IDENT
    in_maps = []
    for g in range(NCORES):
        sl = slice(g * BLOC, (g + 1) * BLOC)
        m = {
            "input_l": input[sl],
            "context_l": context[sl],
            "m1": m1[sl],
            "w_ctx": W_ctx,
            "w_in": W_in,
            "vvec": V,
            "bsum": bsum,
            "ident_i": _IDENT,
        }
        if inf_branch:
            m["m2"] = m2[sl]
        in_maps.append(m)
    return in_maps, inf_branch


def assemble_outputs(res, b_ctx):
    hid, alp = [], []
    for g in range(NCORES):
        r = res.results[g]
        L = r["lsum_o"].sum(axis=1, keepdims=True)          # [BLOC, 1]
        alp.append(r["alpha_o"] / L)
        hid.append(r["hidden_o"] / L + b_ctx[None, :])
    return (np.concatenate(hid, axis=0).astype(np.float32),
            np.concatenate(alp, axis=0).astype(np.float32))


def kernel(**inputs):
    in_maps, inf_branch = make_in_maps(**inputs)
    nc = _get_prog(inf_branch)
    res = run_bass_kernel_spmd(nc, in_maps, core_ids=list(range(NCORES)))
    b_ctx = np.ascontiguousarray(inputs["b_ctx"], dtype=np.float32)
    return assemble_outputs(res, b_ctx)


# revision 14
# speedup vs baseline: 1.0311x; 1.0223x over previous
"""Trainium2 Bass kernel for additive-attention (nn_Attention_77403900609148).

Computation (per batch row b):
    inp  = input @ W_in.T + b_in                      # [H]
    ctx  = W_ctx @ context[b].T + b_ctx               # [H, S]
    att  = V . tanh(inp[:,None] + ctx)                # [S]
    att  = att * alpha_mask[b]   (or -inf mask branch, resolved host-side)
    alpha = softmax(att)                              # [S]
    hidden = ctx @ alpha = W_ctx @ (context[b].T @ alpha) + b_ctx

Device computes exp(att) (softmax without max-subtraction: energies are
bounded ~|att| < 40, exp is safe in fp32) and the unnormalized
y = context.T @ exp(att); the 1/sum(exp) normalization and b_ctx bias are
applied host-side.

Sharding: data-parallel over batch B=64 across 8 NeuronCores (8 rows each);
the small weights are replicated.
"""

import sys

if '/opt/trn_rl_repo' not in sys.path:
    sys.path.insert(0, '/opt/trn_rl_repo')

import numpy as np

import concourse.bass as bass
import concourse.tile as tile
from concourse import mybir, bacc, masks
from concourse.bass_utils import run_bass_kernel_spmd

F32 = mybir.dt.float32
F32R = mybir.dt.float32r
AFT = mybir.ActivationFunctionType

B, S, D, H = 64, 2048, 512, 512
NCORES = 8
BLOC = B // NCORES          # 8 batch rows per core
SC = S // 128               # 16 s-chunks of 128
ST = S // 512               # 4 s-tiles of 512
DC = D // 128               # 4 d-chunks
HC = H // 128               # 4 h-chunks


def build_program(inf_branch: bool):
    nc = bacc.Bacc("TRN2", num_devices=1, debug=False, target_bir_lowering=False)

    # ---- per-core DRAM I/O ----
    d_input = nc.dram_tensor("input_l", [BLOC, D], F32, kind="ExternalInput")
    d_ctx = nc.dram_tensor("context_l", [BLOC, S, D], F32R, kind="ExternalInput")
    d_m1 = nc.dram_tensor("m1", [BLOC, S], F32, kind="ExternalInput")
    if inf_branch:
        d_m2 = nc.dram_tensor("m2", [BLOC, S], F32, kind="ExternalInput")
    d_wctx = nc.dram_tensor("w_ctx", [H, D], F32R, kind="ExternalInput")
    d_win = nc.dram_tensor("w_in", [H, D], F32R, kind="ExternalInput")
    d_v = nc.dram_tensor("vvec", [H], F32R, kind="ExternalInput")
    d_bsum = nc.dram_tensor("bsum", [H], F32, kind="ExternalInput")   # b_in + b_ctx
    d_ident = nc.dram_tensor("ident_i", [128, 128], F32, kind="ExternalInput")

    d_hid = nc.dram_tensor("hidden_o", [BLOC, H], F32, kind="ExternalOutput")
    d_alpha = nc.dram_tensor("alpha_o", [BLOC, S], F32, kind="ExternalOutput")
    d_lsum = nc.dram_tensor("lsum_o", [BLOC, ST], F32, kind="ExternalOutput")

    with tile.TileContext(nc) as tc:
        with (
            tc.tile_pool(name="persist", bufs=1) as pp,
            tc.tile_pool(name="cnat", bufs=2) as cnat_pool,
            tc.tile_pool(name="ctp", bufs=2) as ct_pool,
            tc.tile_pool(name="tpool", bufs=2) as t_pool,
            tc.tile_pool(name="rowp", bufs=2) as row_pool,
            tc.tile_pool(name="ps_main", bufs=2, space="PSUM") as ps_main,
            tc.tile_pool(name="ps_tr", bufs=2, space="PSUM") as ps_tr,
            tc.tile_pool(name="ps_small", bufs=1, space="PSUM") as ps_small,
            tc.tile_pool(name="ps_att", bufs=2, space="PSUM") as ps_att,
            tc.tile_pool(name="ps_y", bufs=1, space="PSUM") as ps_y,
        ):
            # ---------------- setup ----------------
            # DMA order matters: everything the first tanh depends on
            # (ident, W_in, input, biases, W_ctx, context[0]) goes first.
            ident_f = pp.tile([128, 128], F32, tag="ident_f")
            nc.sync.dma_start(ident_f[:], d_ident.ap())
            ident = pp.tile([128, 128], F32R, tag="ident")
            nc.sync.dma_start(ident[:], d_ident.ap().bitcast(F32R))

            sp_ctx = tc.tile_pool(name="setup", bufs=1)
            sp = sp_ctx.__enter__()
            winT = [sp.tile([128, H], F32R, tag=f"winT{j}", name=f"winT{j}")
                    for j in range(DC)]
            wnat_wi = sp.tile([128, HC, D], F32R, tag="wnat", name="wnat_wi")
            nc.sync.dma_start(
                wnat_wi[:], d_win.ap().rearrange("(c p) d -> p c d", p=128))
            in_sb = sp.tile([BLOC, D], F32, tag="in_sb")
            nc.sync.dma_start(in_sb[:], d_input.ap())
            v_sb = pp.tile([128, HC], F32R, tag="v_sb")
            nc.sync.dma_start(v_sb[:], d_v.ap().rearrange("(c p) -> p c", p=128))
            bsum_sb = pp.tile([128, HC], F32, tag="bsum_sb")
            nc.sync.dma_start(bsum_sb[:], d_bsum.ap().rearrange("(c p) -> p c", p=128))

            wctxT = [pp.tile([128, H], F32R, tag=f"wctxT{j}", name=f"wctxT{j}")
                     for j in range(DC)]
            sp2_ctx = tc.tile_pool(name="setup2", bufs=1)
            sp2 = sp2_ctx.__enter__()
            wnat_wc = sp2.tile([128, HC, D], F32R, tag="wnat_wc")
            nc.sync.dma_start(
                wnat_wc[:], d_wctx.ap().rearrange("(c p) d -> p c d", p=128))

            # batch 0 context + m1 loads, queued right behind the weights
            c_nat0 = cnat_pool.tile([128, SC, D], F32R, tag="c_nat", name="c_nat0")
            ctx_b0 = d_ctx.ap()[0].rearrange("(n p) d -> p n d", p=128)
            for q in range(4):
                nc.sync.dma_start(c_nat0[:, q * 4:(q + 1) * 4, :],
                                  ctx_b0[:, q * 4:(q + 1) * 4, :])
            m1_sb0 = row_pool.tile([1, S], F32, tag="m1_sb", name="m1_sb0")
            nc.sync.dma_start(m1_sb0[:], d_m1.ap()[0:1, :])

            # W_in transposes + input_linear (gates the first tanh)
            for j in range(DC):
                pw = ps_tr.tile([128, 512], F32R, tag="ps_tr", name=f"pw_wi{j}")
                for c in range(HC):
                    nc.tensor.transpose(
                        pw[:, c * 128:(c + 1) * 128],
                        wnat_wi[:, c, j * 128:(j + 1) * 128], ident[:])
                nc.vector.tensor_copy(winT[j][:], pw[:])

            p_it = ps_small.tile([128, DC * BLOC], F32, tag="ps_small")
            for j in range(DC):
                nc.tensor.transpose(
                    p_it[:, j * BLOC:(j + 1) * BLOC],
                    in_sb[:, j * 128:(j + 1) * 128], ident_f[:BLOC, :BLOC])
            inputT = sp.tile([128, DC, BLOC], F32R, tag="inputT")
            nc.vector.tensor_copy(
                inputT[:], p_it[:].rearrange("p (j b) -> p j b", j=DC))

            inpb = [pp.tile([128, BLOC], F32, tag=f"inpb{c}", name=f"inpb{c}")
                    for c in range(HC)]
            for c in range(HC):
                p_inp = ps_small.tile([128, BLOC], F32, tag="ps_small")
                for j in range(DC):
                    nc.tensor.matmul(
                        p_inp[:], winT[j][:, c * 128:(c + 1) * 128],
                        inputT[:, j, :], start=(j == 0), stop=(j == DC - 1))
                nc.vector.tensor_scalar_add(
                    inpb[c][:], p_inp[:], bsum_sb[:, c:c + 1])

            # W_ctx transposes
            for j in range(DC):
                pw = ps_tr.tile([128, 512], F32R, tag="ps_tr", name=f"pw_wc{j}")
                for c in range(HC):
                    nc.tensor.transpose(
                        pw[:, c * 128:(c + 1) * 128],
                        wnat_wc[:, c, j * 128:(j + 1) * 128], ident[:])
                nc.vector.tensor_copy(wctxT[j][:], pw[:])
            sp2_ctx.__exit__(None, None, None)
            sp_ctx.__exit__(None, None, None)

            yT_all = pp.tile([128, DC, BLOC], F32R, tag="yT_all")

            # ---------------- per batch ----------------
            for b in range(BLOC):
                if b == 0:
                    c_nat = c_nat0
                    m1_sb = m1_sb0
                else:
                    c_nat = cnat_pool.tile([128, SC, D], F32R, tag="c_nat",
                                           name=f"c_nat{b}")
                    ctx_b = d_ctx.ap()[b].rearrange("(n p) d -> p n d", p=128)
                    for q in range(4):
                        nc.sync.dma_start(c_nat[:, q * 4:(q + 1) * 4, :],
                                          ctx_b[:, q * 4:(q + 1) * 4, :])
                    m1_sb = row_pool.tile([1, S], F32, tag="m1_sb",
                                          name=f"m1_sb{b}")
                    nc.sync.dma_start(m1_sb[:], d_m1.ap()[b:b + 1, :])
                if inf_branch:
                    m2_sb = row_pool.tile([1, S], F32, tag="m2_sb")
                    nc.sync.dma_start(m2_sb[:], d_m2.ap()[b:b + 1, :])

                # transpose context: CT[p=d, (j, s)] ; j = d-chunk
                ct = ct_pool.tile([128, DC, S], F32R, tag="ct")
                for sc in range(SC):
                    ptr = ps_tr.tile([128, 512], F32R, tag="ps_tr")
                    for j in range(DC):
                        nc.tensor.transpose(
                            ptr[:, j * 128:(j + 1) * 128],
                            c_nat[:, sc, j * 128:(j + 1) * 128], ident[:])
                    nc.vector.tensor_copy(
                        ct[:, :, sc * 128:(sc + 1) * 128],
                        ptr[:].rearrange("p (j q) -> p j q", j=DC))

                alpha_row = row_pool.tile([1, S], F32, tag="alpha_row")
                acc4 = row_pool.tile([1, ST], F32, tag="acc4")
                p_y = ps_y.tile([1, D], F32, tag="ps_y")

                p_atts = {}

                def st_head(st):
                    p_att = ps_att.tile([1, 512], F32, tag="ps_att",
                                        name=f"p_att_{b}_{st}")
                    p_atts[st] = p_att
                    for h in range(HC):
                        p_main = ps_main.tile([128, 512], F32, tag="ps_main",
                                              name=f"p_main_{b}_{st}_{h}")
                        for j in range(DC):
                            nc.tensor.matmul(
                                p_main[:], wctxT[j][:, h * 128:(h + 1) * 128],
                                ct[:, j, st * 512:(st + 1) * 512],
                                start=(j == 0), stop=(j == DC - 1))
                        t_sb = t_pool.tile([128, 512], F32R, tag="t_sb",
                                           name=f"t_sb_{b}_{st}_{h}")
                        nc.scalar.activation(
                            t_sb[:], p_main[:], AFT.Tanh, bias=inpb[h][:, b:b + 1])
                        nc.tensor.matmul(
                            p_att[:], v_sb[:, h:h + 1], t_sb[:],
                            start=(h == 0), stop=(h == HC - 1))

                def st_tail(st):
                    p_att = p_atts.pop(st)
                    att2 = row_pool.tile([1, 512], F32, tag="att2",
                                         name=f"att2_{b}_{st}")
                    nc.vector.tensor_mul(
                        att2[:], p_att[:], m1_sb[:, st * 512:(st + 1) * 512])
                    if inf_branch:
                        nc.vector.tensor_add(
                            att2[:], att2[:], m2_sb[:, st * 512:(st + 1) * 512])
                    nc.scalar.activation(
                        alpha_row[:, st * 512:(st + 1) * 512], att2[:], AFT.Exp,
                        accum_out=acc4[:, st:st + 1])
                    p_at4 = ps_small.tile([128, 4], F32, tag="ps_small",
                                          name=f"p_at4_{b}_{st}")
                    for q in range(4):
                        sc = st * 4 + q
                        nc.tensor.transpose(
                            p_at4[:, q:q + 1],
                            alpha_row[:, sc * 128:(sc + 1) * 128],
                            ident_f[:1, :1])
                    alphaT = row_pool.tile([128, 4], F32R, tag="alphaT",
                                           name=f"alphaT_{b}_{st}")
                    nc.vector.tensor_copy(alphaT[:], p_at4[:])
                    for q in range(4):
                        sc = st * 4 + q
                        nc.tensor.matmul(
                            p_y[:], alphaT[:, q:q + 1], c_nat[:, sc, :],
                            start=(sc == 0), stop=(sc == SC - 1))

                for st in range(ST):
                    st_head(st)
                    if st > 0:
                        st_tail(st - 1)
                st_tail(ST - 1)

                # outputs: unnormalized exp(att) row, its partial sums, and y
                nc.sync.dma_start(d_alpha.ap()[b:b + 1, :], alpha_row[:])
                nc.sync.dma_start(d_lsum.ap()[b:b + 1, :], acc4[:])

                y_sb = row_pool.tile([1, D], F32, tag="y_sb", bufs=1)
                nc.vector.tensor_copy(y_sb[:], p_y[:])
                p_yt = ps_small.tile([128, DC], F32, tag="ps_small")
                for j in range(DC):
                    nc.tensor.transpose(
                        p_yt[:, j:j + 1],
                        y_sb[:, j * 128:(j + 1) * 128], ident_f[:1, :1])
                nc.vector.tensor_copy(yT_all[:, :, b], p_yt[:])

            # ---------------- hidden_unnorm = W_ctx @ y ----------------
            hid_ps_out = ps_tr.tile([BLOC, H], F32, tag="ps_tr")
            for c in range(HC):
                p_hid = ps_small.tile([128, BLOC], F32, tag="ps_small")
                for j in range(DC):
                    nc.tensor.matmul(
                        p_hid[:], wctxT[j][:, c * 128:(c + 1) * 128],
                        yT_all[:, j, :], start=(j == 0), stop=(j == DC - 1))
                hidT = row_pool.tile([128, BLOC], F32, tag="hidT", bufs=1)
                nc.vector.tensor_copy(hidT[:], p_hid[:])
                nc.tensor.transpose(
                    hid_ps_out[:, c * 128:(c + 1) * 128], hidT[:], ident_f[:])
            hid_out = row_pool.tile([BLOC, H], F32, tag="hid_out", bufs=1)
            nc.vector.tensor_copy(hid_out[:], hid_ps_out[:])
            nc.sync.dma_start(d_hid.ap(), hid_out[:])

    nc.compile()
    return nc


_PROG_CACHE = {}
_IDENT = np.eye(128, dtype=np.float32)


def _get_prog(inf_branch: bool):
    if inf_branch not in _PROG_CACHE:
        _PROG_CACHE[inf_branch] = build_program(inf_branch)
    return _PROG_CACHE[inf_branch]


def make_in_maps(input, context, alpha_mask, W_in, b_in, W_ctx, b_ctx, V, mask):
    input = np.ascontiguousarray(input, dtype=np.float32)
    context = np.ascontiguousarray(context, dtype=np.float32)
    alpha_mask = np.ascontiguousarray(alpha_mask, dtype=np.float32)
    W_in = np.ascontiguousarray(W_in, dtype=np.float32)
    W_ctx = np.ascontiguousarray(W_ctx, dtype=np.float32)
    V = np.ascontiguousarray(V, dtype=np.float32)
    b_in = np.ascontiguousarray(b_in, dtype=np.float32)
    b_ctx = np.ascontiguousarray(b_ctx, dtype=np.float32)

    mb = np.asarray(mask) != 0
    n_true = int(mb.sum())
    inf_branch = (n_true > 0) and (n_true == S)
    if inf_branch:
        m1 = np.ones_like(alpha_mask)
        m2 = np.where(mb, np.float32(-1e30), np.float32(0.0)).astype(np.float32)
    else:
        m1 = alpha_mask
        m2 = None

    bsum = (b_in + b_ctx).astype(np.float32)
    global _IDENT
    in_maps = []
    for g in range(NCORES):
        sl = slice(g * BLOC, (g + 1) * BLOC)
        m = {
            "input_l": input[sl],
            "context_l": context[sl],
            "m1": m1[sl],
            "w_ctx": W_ctx,
            "w_in": W_in,
            "vvec": V,
            "bsum": bsum,
            "ident_i": _IDENT,
        }
        if inf_branch:
            m["m2"] = m2[sl]
        in_maps.append(m)
    return in_maps, inf_branch


def assemble_outputs(res, b_ctx):
    hid, alp = [], []
    for g in range(NCORES):
        r = res.results[g]
        L = r["lsum_o"].sum(axis=1, keepdims=True)          # [BLOC, 1]
        alp.append(r["alpha_o"] / L)
        hid.append(r["hidden_o"] / L + b_ctx[None, :])
    return (np.concatenate(hid, axis=0).astype(np.float32),
            np.concatenate(alp, axis=0).astype(np.float32))


def kernel(**inputs):
    in_maps, inf_branch = make_in_maps(**inputs)
    nc = _get_prog(inf_branch)
    res = run_bass_kernel_spmd(nc, in_maps, core_ids=list(range(NCORES)))
    b_ctx = np.ascontiguousarray(inputs["b_ctx"], dtype=np.float32)
    return assemble_outputs(res, b_ctx)
